# revision 7
# baseline (speedup 1.0000x reference)
"""Trainium2 Bass kernel for nn_Conv2d_NN (retrieval_knn).

Reference computation (per batch b):
  xf = x.reshape(B, C, T)                       # T = H*W = 4096, C = 32
  xn = xf / ||xf||_2(channel axis)              # cosine-normalize tokens
  sim = clip(xn^T xn, -1, 1)                    # [T, T]
  vals, idx = top_k(sim, 9)                     # per row, sorted desc
  prime[c,t,k] = vals[t,k] * xf[c, idx[t,k]]
  out[o,t] = sum_{c,k} prime[c,t,k] * w[o,c,k] + bias[o]

Sharding: data-parallel over batch, one batch per NeuronCore (8 cores).

The output is extremely sensitive to neighbor selection (one swapped row
out of 4096 costs ~1% rel err), so sim must be computed at full fp32
matmul precision and the top-k scanned on fp32 rows.

Per-core device algorithm (flash-style fused top-k, sim never hits HBM):
  stage A: token norms as ones^T (x*x) PE reduction; 1/norm broadcast to
           all partitions on gpsimd; one DVE multiply -> fp32 xn on all
           4 PE row-groups.
  stage B, per super-block of 4 row blocks (4 x 128 tokens):
    - sim row blocks via 4-way row-group-packed fp32 matmuls (K=32 each)
    - ACT evicts PSUM -> SBUF fp32 sim rows
    - DVE subtracts 3*I from the diagonal block (self-sim -> -2)
    - DVE max / max_index -> top-8 values + columns per row
    - slot 0 = self (val 1.0, idx = row token): top-9 assembled
    - gpsimd: ap_gather pulls the 4*9*128 neighbor feature columns (fp32),
      partition_broadcast replicates the vals row, one tensor_tensor
      multiplies them (fp16 out) -- the whole output prep stays on the
      otherwise-idle gpsimd engine
    - conv contraction = 9 accumulating [32x32]x[32x512] fp16 matmuls
      + bias on ACT, DMA out.

Gather column order: j = (q*36 + r*9 + k)*16 + pp where the token is
p = pp*8 + q of row block r (q in [0,8), pp in [0,16)) and k is the
neighbor slot.  This is ap_gather's natural wrapped index order, builds
from idx16 [128, 36] with one DMA per 16-partition replica, and keeps
each k-slice of the gathered matrix an affine matmul access pattern
whose walk order is exactly super-block token order.
"""

import sys

if "/opt/trn_rl_repo" not in sys.path:
    sys.path.insert(0, "/opt/trn_rl_repo")

import numpy as np

B, C, H, W = 8, 32, 64, 64
T = H * W          # 4096
KNN = 9            # neighbors
NCORES = 8
RBS = 128          # row-block size (tokens per block)
NRB = T // RBS     # 32
SUP = 4            # row blocks per super-block
NSUP = NRB // SUP  # 8
SBS = SUP * RBS    # 512 tokens per super-block
CBS = 512          # col-block size (matmul moving dim)
NCB = T // CBS     # 8
O = 32             # conv output channels
RK = SUP * KNN     # 36 (row-block, k) pairs per token-slot group
NI = RBS * RK      # 4608 gathered columns per super-block

_CACHE = {}


def _build_program():
    import concourse.bass as bass
    import concourse.bacc as bacc
    import concourse.mybir as mybir
    from concourse.tile import TileContext

    f32 = mybir.dt.float32
    i16 = mybir.dt.int16
    u16 = mybir.dt.uint16
    f16 = mybir.dt.float16

    nc = bacc.Bacc("TRN2", target_bir_lowering=False, debug=False,
                   num_devices=NCORES)

    xb = nc.dram_tensor("xb", [C, T], f32, kind="ExternalInput")
    wf = nc.dram_tensor("wf", [KNN * C, O], f16, kind="ExternalInput")
    bias = nc.dram_tensor("bias", [O, 1], f32, kind="ExternalInput")
    out = nc.dram_tensor("out", [O, T], f32, kind="ExternalOutput")

    AF = mybir.ActivationFunctionType
    ALU = mybir.AluOpType

    with TileContext(nc) as tc:
        with (
            tc.tile_pool(name="const", bufs=1) as cpool,
            tc.tile_pool(name="xdata", bufs=1) as xpool,
        ):
            # 3*I for the diagonal self-sim mask (DVE subtract)
            ident3 = cpool.tile([128, 128], f32)
            nc.gpsimd.memset(ident3[:], 3.0)
            nc.gpsimd.affine_select(
                out=ident3[:], in_=ident3[:], pattern=[[-1, 128]],
                channel_multiplier=1, base=0, compare_op=ALU.is_equal,
                fill=0.0)
            # iota4[p, r] = p + r*128 (token id of partition p in row blk r)
            iota4 = cpool.tile([128, SUP], u16)
            nc.gpsimd.iota(iota4[:], pattern=[[RBS, SUP]], base=0,
                           channel_multiplier=1)
            ones32 = cpool.tile([C, 1], f32)
            nc.gpsimd.memset(ones32[:], 1.0)
            wf_sb = []
            for k in range(KNN):
                wf_k = cpool.tile([C, O], f16, name=f"wf_k{k}")
                nc.sync.dma_start(out=wf_k[:],
                                  in_=wf.ap()[k * C:(k + 1) * C, :])
                wf_sb.append(wf_k)
            bias_sb = cpool.tile([O, 1], f32)
            nc.sync.dma_start(out=bias_sb[:], in_=bias.ap())

            # raw x replicated onto all four 32-partition row groups
            # (rows 0-31 double as the fp32 gather source)
            xb_rep = xpool.tile([128, T], f32)
            nc.sync.dma_start(
                out=xb_rep[:],
                in_=xb.ap().to_broadcast([C, T, 4]).rearrange(
                    "c t g -> g c t"))
            # fp32 xn replicated onto all four 32-partition row groups
            xn_rep = xpool.tile([128, T], f32)

            # ---- stage A: inverse norms, normalized + replicated xn ----
            with (
                tc.tile_pool(name="saps", bufs=2, space="PSUM") as saps,
                tc.tile_pool(name="sasb", bufs=1) as sasb,
            ):
                sq = sasb.tile([C, T], f32)
                nc.scalar.activation(sq[:], xb_rep[0:C, :], AF.Square)
                nrmrow = sasb.tile([1, T], f32)
                for j in range(NCB):
                    sl = slice(j * CBS, (j + 1) * CBS)
                    nsq_ps = saps.tile([1, CBS], f32, tag="nsq")
                    nc.tensor.matmul(nsq_ps[:], lhsT=ones32[:],
                                     rhs=sq[:, sl], start=True, stop=True)
                    nc.scalar.activation(nrmrow[:, sl], nsq_ps[:], AF.Sqrt)
                rinvrow = sasb.tile([1, T], f32)
                nc.vector.reciprocal(rinvrow[:], nrmrow[:])
                rinv_bc = sasb.tile([128, T], f32)
                nc.gpsimd.partition_broadcast(rinv_bc[:], rinvrow[:],
                                              channels=128)
                for j in range(4):
                    sl = slice(j * 1024, (j + 1) * 1024)
                    nc.vector.tensor_tensor(
                        out=xn_rep[:, sl], in0=xb_rep[:, sl],
                        in1=rinv_bc[:, sl], op=ALU.mult)

            # ---- stage B: fused sim + top-k + gather + conv ----
            with (
                tc.tile_pool(name="simps", bufs=2, space="PSUM") as simps,
                tc.tile_pool(name="ops", bufs=2, space="PSUM") as ops,
                tc.tile_pool(name="row", bufs=2) as rowpool,
                tc.tile_pool(name="small", bufs=4) as spool,
                tc.tile_pool(name="big", bufs=1) as bpool,
                tc.tile_pool(name="big2", bufs=2) as bpool2,
            ):
                def emit_out_stage(psb, vals9, idx16):
                    # ---- wrapped index tile for ap_gather ----
                    idxw = spool.tile([32, NI // 16], i16, tag="idxw")
                    for gr in range(2):
                        nc.gpsimd.dma_start(
                            out=idxw[gr * 16:(gr + 1) * 16, :].rearrange(
                                "pp (q rk) -> pp q rk", q=8),
                            in_=idx16[:].bitcast(i16))
                    # vals row (fp32): vrow[0, p*36 + rk] = vals9[p, rk]
                    vrow = bpool.tile([1, NI], f32, tag="vrow")
                    nc.gpsimd.dma_start(out=vrow[:], in_=vals9[:])
                    # broadcast to the 32 channel partitions on gpsimd
                    vbc = bpool.tile([C, NI], f32, tag="vbc")
                    nc.gpsimd.partition_broadcast(vbc[:], vrow[:],
                                                  channels=C)
                    # ---- gather + scale ----
                    gg = bpool.tile([C, NI], f32, tag="gg")
                    nc.gpsimd.ap_gather(
                        out_ap=gg[:].rearrange("p (n d) -> p n d", d=1),
                        in_ap=xb_rep[0:C, :].rearrange("p (n d) -> p n d",
                                                       d=1),
                        idxs_ap=idxw[:],
                        channels=32, num_elems=T, d=1, num_idxs=NI)
                    pp_t = bpool2.tile([C, NI], f16, tag="pp_t")
                    # vbc holds vals in p-major order m = pp*288 + q*36 + rk;
                    # walk it in gather j-order j = (q*36 + rk)*16 + pp
                    nc.gpsimd.tensor_tensor(
                        out=pp_t[:], in0=gg[:],
                        in1=vbc[:].rearrange("c (pp q rk) -> c q rk pp",
                                             pp=16, q=8),
                        op=ALU.mult)
                    # ---- conv contraction ----
                    out_ps = ops.tile([O, SBS], f32, tag="out_ps")
                    # per-k view, walk (r, pp, q) == super-block token order
                    pview = pp_t[:].rearrange(
                        "c (q r k pp) -> c k r pp q", q=8, r=SUP, k=KNN)
                    for k in range(KNN):
                        nc.tensor.matmul(out_ps[:], lhsT=wf_sb[k][:],
                                         rhs=pview[:, k],
                                         start=(k == 0), stop=(k == KNN - 1))
                    out_sb = spool.tile([O, SBS], f32, tag="out_sb")
                    nc.scalar.activation(out_sb[:], out_ps[:], AF.Identity,
                                         bias=bias_sb[:])
                    nc.scalar.dma_start(
                        out=out.ap()[:, psb * SBS:(psb + 1) * SBS],
                        in_=out_sb[:])

                pending = []
                for sb in range(NSUP):
                    vals9 = spool.tile([RBS, RK], f32, tag="vals9")
                    idx16 = spool.tile([RBS, RK], u16, tag="idx16")
                    v3 = vals9[:].rearrange("p (r k) -> p r k", r=SUP)
                    i3 = idx16[:].rearrange("p (r k) -> p r k", r=SUP)
                    nc.gpsimd.memset(v3[:, :, 0:1], 1.0)
                    nc.gpsimd.tensor_scalar_add(
                        i3[:, :, 0:1],
                        iota4[:].rearrange("p (r one) -> p r one", one=1),
                        sb * SBS)
                    for r in range(SUP):
                        rb = sb * SUP + r
                        rs = slice(rb * RBS, (rb + 1) * RBS)
                        simrow = rowpool.tile([RBS, T], f32, tag="simrow")
                        # 2 quads of 4-way row-group-packed fp32 matmuls;
                        # each [128,1024] psum tile holds 2 col blocks
                        for half in range(2):
                            for j in range(2):
                                ps = simps.tile([RBS, 2 * CBS], f32,
                                                tag="ps", name="ps")
                                for gi in range(2):
                                    g = 2 * j + gi
                                    cb = half * 4 + 2 * j + gi
                                    cs2 = slice(cb * CBS, (cb + 1) * CBS)
                                    nc.tensor.matmul(
                                        ps[:, gi * CBS:(gi + 1) * CBS],
                                        lhsT=xn_rep[32 * g:32 * (g + 1), rs],
                                        rhs=xn_rep[32 * g:32 * (g + 1), cs2],
                                        tile_position=(32 * g, 0),
                                        start=True, stop=True,
                                        skip_group_check=True)
                                c0 = (half * 4 + 2 * j) * CBS
                                nc.scalar.activation(
                                    simrow[:, c0:c0 + 2 * CBS], ps[:],
                                    AF.Copy)
                        # self-sim -> -2 via DVE subtract of 3*I
                        nc.vector.tensor_tensor(
                            out=simrow[:, rs], in0=simrow[:, rs],
                            in1=ident3[:], op=ALU.subtract)
                        nc.vector.max(out=v3[:, r, 1:KNN], in_=simrow[:])
                        nc.vector.max_index(
                            out=i3[:, r, 1:KNN],
                            in_max=v3[:, r, 1:KNN], in_values=simrow[:])
                    pending.append((sb, vals9, idx16))
                    if len(pending) > 1:
                        emit_out_stage(*pending.pop(0))
                for st in pending:
                    emit_out_stage(*st)
    nc.compile()
    return nc


def _get_program():
    if "nc" not in _CACHE:
        _CACHE["nc"] = _build_program()
    return _CACHE["nc"]


def _prep_inputs(x, weight, bias):
    xf = np.ascontiguousarray(np.asarray(x, dtype=np.float32).reshape(B, C, T))
    # wf[(k,c), o] = weight[o, c, k]
    wfm = np.ascontiguousarray(
        np.asarray(weight, dtype=np.float32).transpose(2, 1, 0).reshape(
            KNN * C, O).astype(np.float16))
    bp = np.ascontiguousarray(np.asarray(bias, dtype=np.float32).reshape(O, 1))
    return [
        {"xb": np.ascontiguousarray(xf[b]), "wf": wfm, "bias": bp}
        for b in range(B)
    ]


def kernel(x, weight, bias):
    from concourse import bass_utils

    nc = _get_program()
    in_maps = _prep_inputs(x, weight, bias)
    res = bass_utils.run_bass_kernel_spmd(nc, in_maps,
                                          core_ids=list(range(NCORES)))
    out = np.stack([res.results[b]["out"] for b in range(B)])
    return np.ascontiguousarray(out.reshape(B, O, H, W).astype(np.float32))


# revision 8
# speedup vs baseline: 1.1559x; 1.1559x over previous
"""Trainium2 Bass kernel for nn_Conv2d_NN (retrieval_knn).

Reference computation (per batch b):
  xf = x.reshape(B, C, T)                       # T = H*W = 4096, C = 32
  xn = xf / ||xf||_2(channel axis)              # cosine-normalize tokens
  sim = clip(xn^T xn, -1, 1)                    # [T, T]
  vals, idx = top_k(sim, 9)                     # per row, sorted desc
  prime[c,t,k] = vals[t,k] * xf[c, idx[t,k]]
  out[o,t] = sum_{c,k} prime[c,t,k] * w[o,c,k] + bias[o]

Sharding: data-parallel over batch, one batch per NeuronCore (8 cores).

The output is extremely sensitive to neighbor selection (one swapped row
out of 4096 costs ~1% rel err), so sim must be computed at full fp32
matmul precision and the top-k scanned on fp32 rows.

Per-core device algorithm (flash-style fused top-k, sim never hits HBM):
  stage A: token norms as ones^T (x*x) PE reduction; 1/norm broadcast to
           all partitions on gpsimd; one DVE multiply -> fp32 xn on all
           4 PE row-groups.
  stage B, per super-block of 4 row blocks (4 x 128 tokens):
    - sim row blocks via 4-way row-group-packed fp32 matmuls (K=32 each)
    - ACT evicts PSUM -> SBUF fp32 sim rows
    - DVE subtracts 3*I from the diagonal block (self-sim -> -2)
    - DVE max / max_index -> top-8 values + columns per row
    - slot 0 = self (val 1.0, idx = row token): top-9 assembled
    - gpsimd: ap_gather pulls the 4*9*128 neighbor feature columns (fp32),
      partition_broadcast replicates the vals row, one tensor_tensor
      multiplies them (fp16 out) -- the whole output prep stays on the
      otherwise-idle gpsimd engine
    - conv contraction = 9 accumulating [32x32]x[32x512] fp16 matmuls
      + bias on ACT, DMA out.

Gather column order: j = (q*36 + r*9 + k)*16 + pp where the token is
p = pp*8 + q of row block r (q in [0,8), pp in [0,16)) and k is the
neighbor slot.  This is ap_gather's natural wrapped index order, builds
from idx16 [128, 36] with one DMA per 16-partition replica, and keeps
each k-slice of the gathered matrix an affine matmul access pattern
whose walk order is exactly super-block token order.
"""

import sys

if "/opt/trn_rl_repo" not in sys.path:
    sys.path.insert(0, "/opt/trn_rl_repo")

import numpy as np

B, C, H, W = 8, 32, 64, 64
T = H * W          # 4096
KNN = 9            # neighbors
NCORES = 8
RBS = 128          # row-block size (tokens per block)
NRB = T // RBS     # 32
SUP = 4            # row blocks per super-block
NSUP = NRB // SUP  # 8
SBS = SUP * RBS    # 512 tokens per super-block
CBS = 512          # col-block size (matmul moving dim)
NCB = T // CBS     # 8
O = 32             # conv output channels
RK = SUP * KNN     # 36 (row-block, k) pairs per token-slot group
NI = RBS * RK      # 4608 gathered columns per super-block

_CACHE = {}


def _build_program():
    import concourse.bass as bass
    import concourse.bacc as bacc
    import concourse.mybir as mybir
    from concourse.tile import TileContext

    f32 = mybir.dt.float32
    i16 = mybir.dt.int16
    u16 = mybir.dt.uint16
    f16 = mybir.dt.float16

    nc = bacc.Bacc("TRN2", target_bir_lowering=False, debug=False,
                   num_devices=NCORES)

    xb = nc.dram_tensor("xb", [C, T], f32, kind="ExternalInput")
    wf = nc.dram_tensor("wf", [KNN * C, O], f16, kind="ExternalInput")
    bias = nc.dram_tensor("bias", [O, 1], f32, kind="ExternalInput")
    out = nc.dram_tensor("out", [O, T], f32, kind="ExternalOutput")
    vrow_dram = nc.dram_tensor("vrow_dram", [1, NI], f16, kind="Internal")
    rinv_dram = nc.dram_tensor("rinv_dram", [1, T], f32, kind="Internal")

    AF = mybir.ActivationFunctionType
    ALU = mybir.AluOpType

    with TileContext(nc) as tc:
        with (
            tc.tile_pool(name="const", bufs=1) as cpool,
            tc.tile_pool(name="xdata", bufs=1) as xpool,
        ):
            # 3*I for the diagonal self-sim mask (DVE subtract)
            ident3 = cpool.tile([128, 128], f32)
            nc.gpsimd.memset(ident3[:], 3.0)
            nc.gpsimd.affine_select(
                out=ident3[:], in_=ident3[:], pattern=[[-1, 128]],
                channel_multiplier=1, base=0, compare_op=ALU.is_equal,
                fill=0.0)
            # iota4[p, r] = p + r*128 (token id of partition p in row blk r)
            iota4 = cpool.tile([128, SUP], u16)
            nc.gpsimd.iota(iota4[:], pattern=[[RBS, SUP]], base=0,
                           channel_multiplier=1)
            ones32 = cpool.tile([C, 1], f32)
            nc.gpsimd.memset(ones32[:], 1.0)
            wf_sb = []
            for k in range(KNN):
                wf_k = cpool.tile([C, O], f16, name=f"wf_k{k}")
                nc.sync.dma_start(out=wf_k[:],
                                  in_=wf.ap()[k * C:(k + 1) * C, :])
                wf_sb.append(wf_k)
            bias_sb = cpool.tile([O, 1], f32)
            nc.sync.dma_start(out=bias_sb[:], in_=bias.ap())

            # raw x replicated onto all four 32-partition row groups
            # (rows 0-31 double as the fp32 gather source)
            xb_rep = xpool.tile([128, T], f32)
            nc.sync.dma_start(
                out=xb_rep[:],
                in_=xb.ap().to_broadcast([C, T, 4]).rearrange(
                    "c t g -> g c t"))
            # fp32 xn replicated onto all four 32-partition row groups
            xn_rep = xpool.tile([128, T], f32)

            # ---- stage A: inverse norms, normalized + replicated xn ----
            with (
                tc.tile_pool(name="saps", bufs=2, space="PSUM") as saps,
                tc.tile_pool(name="sasb", bufs=1) as sasb,
            ):
                sq = sasb.tile([C, T], f32)
                nc.scalar.activation(sq[:], xb_rep[0:C, :], AF.Square)
                nrmrow = sasb.tile([1, T], f32)
                for j in range(NCB):
                    sl = slice(j * CBS, (j + 1) * CBS)
                    nsq_ps = saps.tile([1, CBS], f32, tag="nsq")
                    nc.tensor.matmul(nsq_ps[:], lhsT=ones32[:],
                                     rhs=sq[:, sl], start=True, stop=True)
                    nc.scalar.activation(nrmrow[:, sl], nsq_ps[:], AF.Sqrt)
                rinvrow = sasb.tile([1, T], f32)
                nc.vector.reciprocal(rinvrow[:], nrmrow[:])
                nc.sync.dma_start(out=rinv_dram.ap(), in_=rinvrow[:])
                rinv_bc = sasb.tile([128, T], f32)
                nc.sync.dma_start(
                    out=rinv_bc[:],
                    in_=rinv_dram.ap().to_broadcast([1, T, 128]).rearrange(
                        "one t g -> g (one t)"))
                for j in range(4):
                    sl = slice(j * 1024, (j + 1) * 1024)
                    nc.vector.tensor_tensor(
                        out=xn_rep[:, sl], in0=xb_rep[:, sl],
                        in1=rinv_bc[:, sl], op=ALU.mult)

            # ---- stage B: fused sim + top-k + gather + conv ----
            with (
                tc.tile_pool(name="simps", bufs=2, space="PSUM") as simps,
                tc.tile_pool(name="ops", bufs=2, space="PSUM") as ops,
                tc.tile_pool(name="row", bufs=2) as rowpool,
                tc.tile_pool(name="small", bufs=4) as spool,
                tc.tile_pool(name="big", bufs=1) as bpool,
                tc.tile_pool(name="big2", bufs=2) as bpool2,
            ):
                def emit_out_stage(psb, vals9h, idx16):
                    # ---- wrapped index tile for ap_gather ----
                    idxw = spool.tile([32, NI // 16], i16, tag="idxw")
                    for gr in range(2):
                        nc.sync.dma_start(
                            out=idxw[gr * 16:(gr + 1) * 16, :].rearrange(
                                "pp (q rk) -> pp q rk", q=8),
                            in_=idx16[:].bitcast(i16))
                    # vals row (fp16): vrow_dram[0, p*36 + rk] = vals9h[p, rk]
                    # bounced through DRAM to partition-broadcast it (the
                    # sync DMA queue is in-order, so write-then-read is safe)
                    nc.sync.dma_start(out=vrow_dram.ap(), in_=vals9h[:])
                    vbc = bpool.tile([C, NI], f16, tag="vbc")
                    nc.sync.dma_start(
                        out=vbc[:],
                        in_=vrow_dram.ap().to_broadcast(
                            [1, NI, C]).rearrange("one n g -> g (one n)"))
                    # ---- gather (the only gpsimd op: no ucode lib thrash)
                    gg = bpool.tile([C, NI], f32, tag="gg")
                    nc.gpsimd.ap_gather(
                        out_ap=gg[:].rearrange("p (n d) -> p n d", d=1),
                        in_ap=xb_rep[0:C, :].rearrange("p (n d) -> p n d",
                                                       d=1),
                        idxs_ap=idxw[:],
                        channels=32, num_elems=T, d=1, num_idxs=NI)
                    pp_t = bpool2.tile([C, NI], f16, tag="pp_t")
                    # vbc holds vals in p-major order m = pp*288 + q*36 + rk;
                    # walk it in gather j-order j = (q*36 + rk)*16 + pp
                    nc.vector.tensor_tensor(
                        out=pp_t[:], in0=gg[:],
                        in1=vbc[:].rearrange("c (pp q rk) -> c q rk pp",
                                             pp=16, q=8),
                        op=ALU.mult)
                    # ---- conv contraction ----
                    out_ps = ops.tile([O, SBS], f32, tag="out_ps")
                    # per-k view, walk (r, pp, q) == super-block token order
                    pview = pp_t[:].rearrange(
                        "c (q r k pp) -> c k r pp q", q=8, r=SUP, k=KNN)
                    for k in range(KNN):
                        nc.tensor.matmul(out_ps[:], lhsT=wf_sb[k][:],
                                         rhs=pview[:, k],
                                         start=(k == 0), stop=(k == KNN - 1))
                    out_sb = spool.tile([O, SBS], f32, tag="out_sb")
                    nc.scalar.activation(out_sb[:], out_ps[:], AF.Identity,
                                         bias=bias_sb[:])
                    nc.scalar.dma_start(
                        out=out.ap()[:, psb * SBS:(psb + 1) * SBS],
                        in_=out_sb[:])

                pending = []
                for sb in range(NSUP):
                    vals9 = spool.tile([RBS, RK], f32, tag="vals9")
                    vals9h = spool.tile([RBS, RK], f16, tag="vals9h")
                    idx16 = spool.tile([RBS, RK], u16, tag="idx16")
                    v3 = vals9[:].rearrange("p (r k) -> p r k", r=SUP)
                    vh3 = vals9h[:].rearrange("p (r k) -> p r k", r=SUP)
                    i3 = idx16[:].rearrange("p (r k) -> p r k", r=SUP)
                    nc.vector.memset(vh3[:, :, 0:1], 1.0)
                    nc.vector.tensor_scalar_add(
                        i3[:, :, 0:1],
                        iota4[:].rearrange("p (r one) -> p r one", one=1),
                        sb * SBS)
                    for r in range(SUP):
                        rb = sb * SUP + r
                        rs = slice(rb * RBS, (rb + 1) * RBS)
                        simrow = rowpool.tile([RBS, T], f32, tag="simrow")
                        # 2 quads of 4-way row-group-packed fp32 matmuls;
                        # each [128,1024] psum tile holds 2 col blocks
                        for half in range(2):
                            for j in range(2):
                                ps = simps.tile([RBS, 2 * CBS], f32,
                                                tag="ps", name="ps")
                                for gi in range(2):
                                    g = 2 * j + gi
                                    cb = half * 4 + 2 * j + gi
                                    cs2 = slice(cb * CBS, (cb + 1) * CBS)
                                    nc.tensor.matmul(
                                        ps[:, gi * CBS:(gi + 1) * CBS],
                                        lhsT=xn_rep[32 * g:32 * (g + 1), rs],
                                        rhs=xn_rep[32 * g:32 * (g + 1), cs2],
                                        tile_position=(32 * g, 0),
                                        start=True, stop=True,
                                        skip_group_check=True)
                                c0 = (half * 4 + 2 * j) * CBS
                                nc.scalar.activation(
                                    simrow[:, c0:c0 + 2 * CBS], ps[:],
                                    AF.Copy)
                        # self-sim -> -2 via DVE subtract of 3*I
                        nc.vector.tensor_tensor(
                            out=simrow[:, rs], in0=simrow[:, rs],
                            in1=ident3[:], op=ALU.subtract)
                        nc.vector.max(out=v3[:, r, 1:KNN], in_=simrow[:])
                        nc.vector.max_index(
                            out=i3[:, r, 1:KNN],
                            in_max=v3[:, r, 1:KNN], in_values=simrow[:])
                        nc.vector.tensor_copy(vh3[:, r, 1:KNN],
                                              v3[:, r, 1:KNN])
                    pending.append((sb, vals9h, idx16))
                    if len(pending) > 1:
                        emit_out_stage(*pending.pop(0))
                for st in pending:
                    emit_out_stage(*st)
    nc.compile()
    return nc


def _get_program():
    if "nc" not in _CACHE:
        _CACHE["nc"] = _build_program()
    return _CACHE["nc"]


def _prep_inputs(x, weight, bias):
    xf = np.ascontiguousarray(np.asarray(x, dtype=np.float32).reshape(B, C, T))
    # wf[(k,c), o] = weight[o, c, k]
    wfm = np.ascontiguousarray(
        np.asarray(weight, dtype=np.float32).transpose(2, 1, 0).reshape(
            KNN * C, O).astype(np.float16))
    bp = np.ascontiguousarray(np.asarray(bias, dtype=np.float32).reshape(O, 1))
    return [
        {"xb": np.ascontiguousarray(xf[b]), "wf": wfm, "bias": bp}
        for b in range(B)
    ]


def kernel(x, weight, bias):
    from concourse import bass_utils

    nc = _get_program()
    in_maps = _prep_inputs(x, weight, bias)
    res = bass_utils.run_bass_kernel_spmd(nc, in_maps,
                                          core_ids=list(range(NCORES)))
    out = np.stack([res.results[b]["out"] for b in range(B)])
    return np.ascontiguousarray(out.reshape(B, O, H, W).astype(np.float32))
